# revision 18
# baseline (speedup 1.0000x reference)
"""Trainium2 Bass kernel for nn_MultiHeadAttention_29231547416651.

Sparse bipartite exp-weighted multi-head attention:
  out[b,q,e] = sum_h ( attn_h @ V_h ) @ Wout_h, with
  attn = L1-normalized exp(QK^T/sqrt(kd)) * Wbipartite, masked by (mask != I).

Sharding: pure data-parallel over batch B=8 across the 8 NeuronCores
(one batch element per core). All weight tensors are small and replicated.

Per-core dataflow (python-unrolled, Tile framework schedules engines):
  - qT = q_b^T via PE transposes; QT/KT = W^T @ qT packed over all heads
    ([128=(h,kd), G]); V natural [G, (h,v)] + ones column per head (the ones
    column makes the attention L1 denominator fall out of the heads matmul).
  - P^T[k,q] = (mask^T == I) * W''  (W'' = bipartite weights with w+(w==0),
    1 in diagonal blocks) built once per core in bf16, reused by all heads.
  - per (head, key-chunk): scoresT[k,q] on PE (K=16), exp on ScalarE
    (scale=1/4) PSUM->SBUF bf16, * P^T on VectorE (bf16 2x mode), then
    heads^T accumulation on PE with the V'(17-col) stationary operand.
    Heads 0-6 stack into one [119,1024] PSUM region, head 7 uses the misc
    slot, so evacuation is two wide DVE copies.
  - normalize via reciprocal of the S rows (reshaped [64,128] to keep the
    iterative-divide cheap) + a replicate-matmul, final projection on PE.
"""

import os
import sys

import numpy as np

_TRN_REPO = "/opt/trn_rl_repo"
if _TRN_REPO not in sys.path:
    sys.path.insert(0, _TRN_REPO)

import concourse.bass as bass  # noqa: E402
import concourse.tile as tile  # noqa: E402
from concourse import bacc, bass_utils, mybir  # noqa: E402

B, G, D = 8, 1024, 128
H, KD, VD, E = 8, 16, 16, 128
U = G // 2
NT = G // 128  # 8 row-tiles / key-chunks
HK = H * KD   # 128
HV = H * VD   # 128
F32 = mybir.dt.float32
BF16 = mybir.dt.bfloat16
U8 = mybir.dt.uint8
AF = mybir.ActivationFunctionType
ALU = mybir.AluOpType

NORM = float(1.0 / np.sqrt(np.float32(KD)))


def build_kernel(nc: bass.Bass, reps: int = 1):
    tc_ctx = tile.TileContext(nc)

    qb = nc.dram_tensor("qb", [G, D], F32, kind="ExternalInput").ap()
    maskb = nc.dram_tensor("maskb", [G, G], U8, kind="ExternalInput").ap()
    wb = nc.dram_tensor("wb", [U, U], F32, kind="ExternalInput").ap()
    Wq = nc.dram_tensor("Wq", [H, D, KD], F32, kind="ExternalInput").ap()
    Wk = nc.dram_tensor("Wk", [H, D, KD], F32, kind="ExternalInput").ap()
    Wv = nc.dram_tensor("Wv", [H, D, VD], F32, kind="ExternalInput").ap()
    Wo = nc.dram_tensor("Wo", [H, KD, E], F32, kind="ExternalInput").ap()
    outb = nc.dram_tensor("outb", [G, E], F32, kind="ExternalOutput").ap()

    with tc_ctx as tc:
        for _ in range(reps):
            _emit(tc, qb, maskb, wb, Wq, Wk, Wv, Wo, outb)
    return nc


def _emit(tc, qb, maskb, wb, Wq, Wk, Wv, Wo, outb):
    nc = tc.nc
    from contextlib import ExitStack

    with ExitStack() as ctx:
        const = ctx.enter_context(tc.tile_pool(name="const", bufs=1))
        persist = ctx.enter_context(tc.tile_pool(name="persist", bufs=1))
        work = ctx.enter_context(tc.tile_pool(name="work", bufs=3))
        cpool = ctx.enter_context(tc.tile_pool(name="cpool", bufs=3))
        # heads pass consumes all 8 attnu chunks of a head after the scores
        # pass, so every chunk must stay resident: bufs=9 (one extra for the
        # next head's first chunk).
        apool = ctx.enter_context(tc.tile_pool(name="apool", bufs=9))
        # PSUM budget: 8 banks = 16KB/partition.
        #   psum_s : [128,1024] f32 (4KB) x2 bufs = 4 banks (scores + all prep)
        #   psum_h : [119,1024] f32 (4KB) x1      = 2 banks (heads 0-6)
        #   psum_m : [128,1024] f32 (4KB) x1      = 2 banks (head 7, finale)
        psum_s = ctx.enter_context(
            tc.tile_pool(name="psum_s", bufs=2, space=bass.MemorySpace.PSUM)
        )
        psum_h = ctx.enter_context(
            tc.tile_pool(name="psum_h", bufs=1, space=bass.MemorySpace.PSUM)
        )
        psum_m = ctx.enter_context(
            tc.tile_pool(name="psum_m", bufs=1, space=bass.MemorySpace.PSUM)
        )

        def ps_tile(shape, dtype):
            return psum_s.tile(shape, dtype, tag="ps", name="ps")

        def pm_tile(shape, dtype):
            return psum_m.tile(shape, dtype, tag="pm", name="pm")

        # ---- constants ----
        eye_f = const.tile([128, 128], F32, tag="eye_f")
        nc.gpsimd.memset(eye_f[:], 1.0)
        nc.gpsimd.affine_select(
            eye_f[:], eye_f[:], pattern=[[1, 128]], base=0,
            channel_multiplier=-1, compare_op=ALU.is_equal, fill=0.0,
        )
        eye_b = const.tile([128, 128], BF16, tag="eye_b")
        nc.vector.tensor_copy(eye_b[:], eye_f[:])
        # Brep [8, 128]: Brep[h, 16h+v] = 1  (replicates a per-head row 16x)
        brep = const.tile([8, HV], F32, tag="brep")
        nc.gpsimd.memset(brep[:], 1.0)
        # keep 1 only where 0 <= col - 16*row < 16 (two affine range cuts)
        nc.gpsimd.affine_select(
            brep[:], brep[:], pattern=[[1, HV]], base=0,
            channel_multiplier=-VD, compare_op=ALU.is_ge, fill=0.0,
        )
        nc.gpsimd.affine_select(
            brep[:], brep[:], pattern=[[-1, HV]], base=VD - 1,
            channel_multiplier=VD, compare_op=ALU.is_ge, fill=0.0,
        )

        # ---- load packed projection weights ----
        # Wq/Wk packed with heads padded to 32-partition slots so per-head
        # matmul operands sit at base partitions {0,32,64,96} (PE tiling
        # alignment): group g holds heads 4g..4g+3, head slot m = h%4 at
        # columns/rows [32m, 32m+16), rest zero.
        # head groups: base partitions limited to {0,32,64} -> 3 heads/group
        HGRP = [(0, 1, 2), (3, 4, 5), (6, 7)]
        NG = len(HGRP)
        wqk_grp = []
        for wsrc, nm in ((Wq, "wqa"), (Wk, "wka")):
            grp = []
            for g, heads in enumerate(HGRP):
                nh = len(heads)
                wt = persist.tile([D, 128], F32, tag=f"{nm}{g}", name=f"{nm}{g}")
                nc.gpsimd.memset(wt[:], 0.0)
                nc.sync.dma_start(
                    wt[:].rearrange("p (m s) -> p m s", s=32)[:, 0:nh, 0:KD],
                    wsrc[heads[0] : heads[0] + nh].rearrange("h d k -> d h k"),
                )
                grp.append(wt)
            wqk_grp.append(grp)
        WvAll = persist.tile([D, HV], F32, tag="WvAll")
        WoAll = persist.tile([HV, E], F32, tag="WoAll")
        nc.sync.dma_start(
            WvAll[:].rearrange("p (h v) -> p h v", v=VD),
            Wv.rearrange("h d v -> d h v"),
        )
        nc.sync.dma_start(WoAll[:], Wo.rearrange("h v e -> (h v) e"))

        # ---- qT via PE transposes ----
        qT = persist.tile([D, G], F32, tag="qT")
        for t in range(NT):
            qn = work.tile([128, D], F32, tag="qn")
            nc.sync.dma_start(qn[:], qb[128 * t : 128 * t + 128, :])
            pt = ps_tile([128, 128], F32)
            nc.tensor.transpose(pt[:], qn[:], eye_f[:])
            nc.vector.tensor_copy(qT[:, 128 * t : 128 * t + 128], pt[:])

        # ---- QT/KT projections, padded-head layout: [128=(4m,32), G] x2 ----
        QT32 = []
        KT32 = []
        for dsts, grp in ((QT32, wqk_grp[0]), (KT32, wqk_grp[1])):
            for g in range(NG):
                nm = "qt32" if dsts is QT32 else "kt32"
                dst = persist.tile([128, G], F32, tag=f"{nm}{g}", name=f"{nm}{g}")
                dsts.append(dst)
                for n in range(2):
                    cs = slice(512 * n, 512 * n + 512)
                    pp = ps_tile([128, 512], F32)
                    nc.tensor.matmul(pp[:], grp[g][:], qT[:, cs])
                    nc.vector.tensor_copy(dst[:, cs], pp[:])

        # ---- V natural + ones column, 32-wide head slots: Vp[t] [128, 8*32]
        # cols [32h,32h+16) = V_h, col 32h+16 = 1 (L1-denominator trick),
        # cols [32h+17,32h+32) = 0 so heads matmuls write full 32-row bands.
        vp_tiles = []
        for t in range(NT):
            vp = persist.tile([128, H * 32], BF16, tag=f"vp{t}")
            vp_tiles.append(vp)
            pv = ps_tile([128, HV], F32)
            nc.tensor.matmul(pv[:], qT[:, 128 * t : 128 * t + 128], WvAll[:])
            vp3 = vp[:].rearrange("p (h s) -> p h s", s=32)
            pv3 = pv[:].rearrange("p (h v) -> p h v", v=VD)
            nc.gpsimd.memset(vp3[:, :, VD:32], 0.0)
            nc.gpsimd.memset(vp3[:, :, VD : VD + 1], 1.0)
            nc.vector.tensor_copy(vp3[:, :, 0:VD], pv3[:])

        # ---- w' = w + (w==0) natural & transposed, bf16 ----
        # wpn[p, 512j+c] = w'[128j+p, c];  wpT[p, 512i+c] = w'[c, 128i+p]
        wn = persist.tile([128, 4 * U], F32, tag="wn")
        nc.sync.dma_start(
            wn[:].rearrange("p (j c) -> p j c", c=U),
            wb.rearrange("(j p) c -> p j c", p=128),
        )
        wpn = persist.tile([128, 4 * U], BF16, tag="wpn")
        nc.vector.scalar_tensor_tensor(
            wpn[:], wn[:], 0.0, wn[:], op0=ALU.is_equal, op1=ALU.add
        )
        wpT = persist.tile([128, 4 * U], BF16, tag="wpT")
        for i in range(4):
            for j in range(4):
                pb = ps_tile([128, 128], BF16)
                nc.tensor.transpose(
                    pb[:],
                    wpn[:, 512 * j + 128 * i : 512 * j + 128 * i + 128],
                    eye_b[:],
                )
                # pb[p, c] = w'[128j+c, 128i+p] -> wpT[p, 512i+128j+c]
                nc.vector.tensor_copy(
                    wpT[:, 512 * i + 128 * j : 512 * i + 128 * j + 128], pb[:]
                )

        # ---- mask natural bf16 (gpsimd converts u8->bf16) ----
        mb_tiles = []
        for t in range(NT):
            mu = work.tile([128, G], U8, tag="mu")
            nc.sync.dma_start(mu[:], maskb[128 * t : 128 * t + 128, :])
            mb = persist.tile([128, G], BF16, tag=f"mb{t}")
            mb_tiles.append(mb)
            nc.gpsimd.tensor_copy(mb[:], mu[:])

        # ---- P^T rows: PT[ki] [128, G] bf16 ----
        pt_tiles = []
        for ki in range(NT):
            ptile = persist.tile([128, G], BF16, tag=f"pt{ki}")
            pt_tiles.append(ptile)
            pm = ps_tile([128, G], BF16)
            for qj in range(NT):
                nc.tensor.transpose(
                    pm[:, 128 * qj : 128 * qj + 128],
                    mb_tiles[qj][:, 128 * ki : 128 * ki + 128],
                    eye_b[:],
                )
            if ki < 4:
                diag = slice(0, U)
                off = slice(U, G)
                wmul = wpT[:, 512 * ki : 512 * ki + 512]
            else:
                diag = slice(U, G)
                off = slice(0, U)
                wmul = wpn[:, 512 * (ki - 4) : 512 * (ki - 4) + 512]
            # diagonal half: (maskT == 0)
            nc.vector.tensor_scalar(
                ptile[:, diag], pm[:, diag], 0.0, None, op0=ALU.is_equal
            )
            # true-diagonal block: (maskT == eye)
            db = slice(128 * ki, 128 * ki + 128)
            nc.vector.scalar_tensor_tensor(
                ptile[:, db], pm[:, db], 0.0, eye_b[:],
                op0=ALU.bypass, op1=ALU.is_equal,
            )
            # off-diagonal half: (maskT == 0) * w'
            nc.vector.scalar_tensor_tensor(
                ptile[:, off], pm[:, off], 0.0, wmul,
                op0=ALU.is_equal, op1=ALU.mult,
            )

        # ---- main attention loop ----
        # Heads grouped 4 per PSUM tile: head slot m accumulates at
        # partitions [32m, 32m+17) (S row at 32m+16). Per head: all scores
        # matmuls first, then all heads matmuls (avoids PE tiling-mode
        # thrash); attnu chunks buffered until the heads pass.
        hstages = []
        for g, heads in enumerate(HGRP):
            phg = psum_h.tile([128, G], F32, tag="ph", name="ph")
            for m, h in enumerate(heads):
                rows = slice(32 * m, 32 * m + 32)
                attnus = []
                for c in range(NT):
                    ps = ps_tile([128, G], F32)
                    lhs_k = KT32[g][:][rows, 128 * c : 128 * c + 128]
                    for n in range(2):
                        cs = slice(512 * n, 512 * n + 512)
                        nc.tensor.matmul(
                            ps[:, cs], lhs_k, QT32[g][:][rows, cs]
                        )
                    compat = cpool.tile([128, G], BF16, tag="compat")
                    nc.scalar.activation(compat[:], ps[:], AF.Exp, scale=NORM)
                    attnu = apool.tile([128, G], BF16, tag="attnu")
                    # one chunk per head on GpSimd to offload the DVE
                    eng = nc.gpsimd if c == 5 else nc.vector
                    eng.tensor_tensor(
                        attnu[:], compat[:], pt_tiles[c][:], ALU.mult
                    )
                    attnus.append(attnu)
                vcol = 32 * h
                orow = slice(32 * m, 32 * m + 32)
                for c in range(NT):
                    vslice = vp_tiles[c][:][:, vcol : vcol + 32]
                    for n in range(2):
                        cs = slice(512 * n, 512 * n + 512)
                        nc.tensor.matmul(
                            phg[:][orow, cs], vslice, attnus[c][:, cs],
                            start=(c == 0), stop=(c == NT - 1),
                        )
            hst = persist.tile([128, G], F32, tag=f"hstage{g}", name=f"hstage{g}")
            hstages.append(hst)
            nrows = 32 * len(heads)
            nc.vector.tensor_copy(hst[0:nrows, :], phg[:][0:nrows, :])

        # ---- gather heads / S, normalize, project ----
        hAllT = persist.tile([HV, G], F32, tag="hAllT")
        for g, heads in enumerate(HGRP):
            for m, h in enumerate(heads):
                prow = 32 * m
                nc.sync.dma_start(
                    hAllT[VD * h : VD * h + VD, :],
                    hstages[g][prow : prow + VD, :],
                )
        # S rows -> rspT[(h,j), p] = S_h[128j+p]  (64 partitions, FD=128)
        rspT = persist.tile([64, 128], F32, tag="rspT")
        for g, heads in enumerate(HGRP):
            for m, h in enumerate(heads):
                srow = 32 * m + VD
                nc.sync.dma_start(
                    rspT[8 * h : 8 * h + 8, :],
                    hstages[g][srow : srow + 1, :],
                )
        rrec = persist.tile([64, 128], F32, tag="rrec")
        nc.vector.reciprocal(rrec[:], rspT[:])
        # back to [8, 1024] rows for the replicate matmul
        rrows = persist.tile([8, G], F32, tag="rrows")
        for h in range(H):
            nc.sync.dma_start(
                rrows[h : h + 1, :], rrec[8 * h : 8 * h + 8, :]
            )
        pr = pm_tile([HV, G], F32)
        for n in range(2):
            cs = slice(512 * n, 512 * n + 512)
            nc.tensor.matmul(pr[:, cs], brep[:], rrows[:, cs])
        hNorm = persist.tile([HV, G], F32, tag="hNorm")
        nc.vector.tensor_tensor(hNorm[:], hAllT[:], pr[:], ALU.mult)
        ostage = persist.tile([128, E], F32, tag="ostage")
        for t in range(NT):
            po = pm_tile([128, E], F32)
            nc.tensor.matmul(po[:], hNorm[:, 128 * t : 128 * t + 128], WoAll[:])
            ost = work.tile([128, E], F32, tag="ost")
            nc.vector.tensor_copy(ost[:], po[:])
            nc.sync.dma_start(outb[128 * t : 128 * t + 128, :], ost[:])
        del ostage


_CACHED = {}


def _get_nc(reps: int = 1):
    key = f"nc{reps}"
    if key not in _CACHED:
        nc = bacc.Bacc("TRN2", target_bir_lowering=False, debug=False)
        build_kernel(nc, reps)
        nc.compile()
        _CACHED[key] = nc
    return _CACHED[key]


def kernel(**inputs: np.ndarray) -> np.ndarray:
    q = np.ascontiguousarray(inputs["q"], dtype=np.float32)
    mask = np.ascontiguousarray(inputs["mask"]).astype(np.uint8)
    weights = np.ascontiguousarray(inputs["weights"], dtype=np.float32)
    shared = {
        "Wq": np.ascontiguousarray(inputs["W_query"], dtype=np.float32),
        "Wk": np.ascontiguousarray(inputs["W_key"], dtype=np.float32),
        "Wv": np.ascontiguousarray(inputs["W_val"], dtype=np.float32),
        "Wo": np.ascontiguousarray(inputs["W_out"], dtype=np.float32),
    }
    nc = _get_nc()
    in_maps = [
        {"qb": q[b], "maskb": mask[b], "wb": weights[b], **shared}
        for b in range(B)
    ]
    res = bass_utils.run_bass_kernel_spmd(
        nc, in_maps, core_ids=list(range(B)),
        trace=bool(int(os.environ.get("KERNEL_TRACE", "0"))),
    )
    out = np.stack([np.asarray(r["outb"]) for r in res.results])
    kernel.last_results = res
    return out.astype(np.float32)


if __name__ == "__main__":
    rng = np.random.default_rng(0)
    inputs = {
        "q": rng.standard_normal((B, G, D), dtype=np.float32),
        "mask": rng.integers(0, 2, (B, G, G)).astype(bool),
        "weights": rng.random((B, U, U), dtype=np.float32),
        "W_query": rng.random((H, D, KD), dtype=np.float32) - 0.5,
        "W_key": rng.random((H, D, KD), dtype=np.float32) - 0.5,
        "W_val": rng.random((H, D, VD), dtype=np.float32) - 0.5,
        "W_out": rng.random((H, KD, E), dtype=np.float32) - 0.5,
    }
    out = kernel(**inputs)
    print(out.shape, out.dtype)


# revision 19
# speedup vs baseline: 1.1623x; 1.1623x over previous
"""Trainium2 Bass kernel for nn_MultiHeadAttention_29231547416651.

Sparse bipartite exp-weighted multi-head attention:
  out[b,q,e] = sum_h ( attn_h @ V_h ) @ Wout_h, with
  attn = L1-normalized exp(QK^T/sqrt(kd)) * Wbipartite, masked by (mask != I).

Sharding: pure data-parallel over batch B=8 across the 8 NeuronCores
(one batch element per core). All weight tensors are small and replicated.

Per-core dataflow (python-unrolled, Tile framework schedules engines):
  - qT = q_b^T via PE transposes; QT/KT = W^T @ qT packed over all heads
    ([128=(h,kd), G]); V natural [G, (h,v)] + ones column per head (the ones
    column makes the attention L1 denominator fall out of the heads matmul).
  - P^T[k,q] = (mask^T == I) * W''  (W'' = bipartite weights with w+(w==0),
    1 in diagonal blocks) built once per core in bf16, reused by all heads.
  - per (head, key-chunk): scoresT[k,q] on PE (K=16), exp on ScalarE
    (scale=1/4) PSUM->SBUF bf16, * P^T on VectorE (bf16 2x mode), then
    heads^T accumulation on PE with the V'(17-col) stationary operand.
    Heads 0-6 stack into one [119,1024] PSUM region, head 7 uses the misc
    slot, so evacuation is two wide DVE copies.
  - normalize via reciprocal of the S rows (reshaped [64,128] to keep the
    iterative-divide cheap) + a replicate-matmul, final projection on PE.
"""

import os
import sys

import numpy as np

_TRN_REPO = "/opt/trn_rl_repo"
if _TRN_REPO not in sys.path:
    sys.path.insert(0, _TRN_REPO)

import concourse.bass as bass  # noqa: E402
import concourse.tile as tile  # noqa: E402
from concourse import bacc, bass_utils, mybir  # noqa: E402

B, G, D = 8, 1024, 128
H, KD, VD, E = 8, 16, 16, 128
U = G // 2
NT = G // 128  # 8 row-tiles / key-chunks
HK = H * KD   # 128
HV = H * VD   # 128
F32 = mybir.dt.float32
BF16 = mybir.dt.bfloat16
U8 = mybir.dt.uint8
AF = mybir.ActivationFunctionType
ALU = mybir.AluOpType

NORM = float(1.0 / np.sqrt(np.float32(KD)))


def build_kernel(nc: bass.Bass, reps: int = 1):
    tc_ctx = tile.TileContext(nc)

    qb = nc.dram_tensor("qb", [G, D], F32, kind="ExternalInput").ap()
    maskb = nc.dram_tensor("maskb", [G, G], U8, kind="ExternalInput").ap()
    wb = nc.dram_tensor("wb", [U, U], F32, kind="ExternalInput").ap()
    # weights pre-packed host-side (pure layout prep, ~256KB total):
    # wqp/wkp: [3, 128, 128] padded head-group tiles; wvp: [128,(h v)];
    # wop: [(h v), 128]
    Wq = nc.dram_tensor("wqp", [3, D, 128], F32, kind="ExternalInput").ap()
    Wk = nc.dram_tensor("wkp", [3, D, 128], F32, kind="ExternalInput").ap()
    Wv = nc.dram_tensor("wvp", [D, HV], F32, kind="ExternalInput").ap()
    Wo = nc.dram_tensor("wop", [HV, E], F32, kind="ExternalInput").ap()
    outb = nc.dram_tensor("outb", [G, E], F32, kind="ExternalOutput").ap()

    with tc_ctx as tc:
        for _ in range(reps):
            _emit(tc, qb, maskb, wb, Wq, Wk, Wv, Wo, outb)
    return nc


def _emit(tc, qb, maskb, wb, Wq, Wk, Wv, Wo, outb):
    nc = tc.nc
    from contextlib import ExitStack

    with ExitStack() as ctx:
        const = ctx.enter_context(tc.tile_pool(name="const", bufs=1))
        persist = ctx.enter_context(tc.tile_pool(name="persist", bufs=1))
        work = ctx.enter_context(tc.tile_pool(name="work", bufs=3))
        cpool = ctx.enter_context(tc.tile_pool(name="cpool", bufs=3))
        # heads pass consumes all 8 attnu chunks of a head after the scores
        # pass, so every chunk must stay resident: bufs=9 (one extra for the
        # next head's first chunk).
        apool = ctx.enter_context(tc.tile_pool(name="apool", bufs=9))
        # PSUM budget: 8 banks = 16KB/partition.
        #   psum_s : [128,1024] f32 (4KB) x2 bufs = 4 banks (scores + all prep)
        #   psum_h : [119,1024] f32 (4KB) x1      = 2 banks (heads 0-6)
        #   psum_m : [128,1024] f32 (4KB) x1      = 2 banks (head 7, finale)
        psum_s = ctx.enter_context(
            tc.tile_pool(name="psum_s", bufs=2, space=bass.MemorySpace.PSUM)
        )
        psum_h = ctx.enter_context(
            tc.tile_pool(name="psum_h", bufs=1, space=bass.MemorySpace.PSUM)
        )
        psum_m = ctx.enter_context(
            tc.tile_pool(name="psum_m", bufs=1, space=bass.MemorySpace.PSUM)
        )

        def ps_tile(shape, dtype):
            return psum_s.tile(shape, dtype, tag="ps", name="ps")

        def pm_tile(shape, dtype):
            return psum_m.tile(shape, dtype, tag="pm", name="pm")

        # ---- constants ----
        eye_f = const.tile([128, 128], F32, tag="eye_f")
        nc.gpsimd.memset(eye_f[:], 1.0)
        nc.gpsimd.affine_select(
            eye_f[:], eye_f[:], pattern=[[1, 128]], base=0,
            channel_multiplier=-1, compare_op=ALU.is_equal, fill=0.0,
        )
        eye_b = const.tile([128, 128], BF16, tag="eye_b")
        nc.vector.tensor_copy(eye_b[:], eye_f[:])
        # Brep [8, 128]: Brep[h, 16h+v] = 1  (replicates a per-head row 16x)
        brep = const.tile([8, HV], F32, tag="brep")
        nc.gpsimd.memset(brep[:], 1.0)
        # keep 1 only where 0 <= col - 16*row < 16 (two affine range cuts)
        nc.gpsimd.affine_select(
            brep[:], brep[:], pattern=[[1, HV]], base=0,
            channel_multiplier=-VD, compare_op=ALU.is_ge, fill=0.0,
        )
        nc.gpsimd.affine_select(
            brep[:], brep[:], pattern=[[-1, HV]], base=VD - 1,
            channel_multiplier=VD, compare_op=ALU.is_ge, fill=0.0,
        )

        # ---- load packed projection weights ----
        # Wq/Wk packed with heads padded to 32-partition slots so per-head
        # matmul operands sit at base partitions {0,32,64,96} (PE tiling
        # alignment): group g holds heads 4g..4g+3, head slot m = h%4 at
        # columns/rows [32m, 32m+16), rest zero.
        # head groups: base partitions limited to {0,32,64} -> 3 heads/group
        HGRP = [(0, 1, 2), (3, 4, 5), (6, 7)]
        NG = len(HGRP)
        wq_all = persist.tile([D, NG * 128], F32, tag="wq_all")
        wk_all = persist.tile([D, NG * 128], F32, tag="wk_all")
        nc.sync.dma_start(
            wq_all[:].rearrange("p (g c) -> p g c", c=128),
            Wq.rearrange("g d c -> d g c"),
        )
        nc.sync.dma_start(
            wk_all[:].rearrange("p (g c) -> p g c", c=128),
            Wk.rearrange("g d c -> d g c"),
        )
        wqk_grp = [
            [wq_all[:][:, 128 * g : 128 * g + 128] for g in range(NG)],
            [wk_all[:][:, 128 * g : 128 * g + 128] for g in range(NG)],
        ]
        WvAll = persist.tile([D, HV], F32, tag="WvAll")
        WoAll = persist.tile([HV, E], F32, tag="WoAll")
        nc.sync.dma_start(WvAll[:], Wv)
        nc.sync.dma_start(WoAll[:], Wo)

        # ---- qT via one blocked DMA (128 descriptors) + PE transposes ----
        # qblk[p, 128t+d] = q[8p+t, d]; transpose of block t gives
        # qT[d, 8p+t], written with a stride-8 free AP.
        qblk = persist.tile([128, G], F32, tag="qblk")
        nc.sync.dma_start(qblk[:], qb.rearrange("(p t) d -> p (t d)", t=8))
        qT = persist.tile([D, G], F32, tag="qT")
        qTv = qT[:].rearrange("d (p t) -> d t p", t=8)
        for t in range(NT):
            pt = ps_tile([128, 128], F32)
            nc.tensor.transpose(pt[:], qblk[:, 128 * t : 128 * t + 128], eye_f[:])
            nc.vector.tensor_copy(qTv[:, t, :], pt[:])

        # ---- QT/KT projections, padded-head layout: [128=(4m,32), G] x2 ----
        QT32 = []
        KT32 = []
        for dsts, grp in ((QT32, wqk_grp[0]), (KT32, wqk_grp[1])):
            for g in range(NG):
                nm = "qt32" if dsts is QT32 else "kt32"
                dst = persist.tile([128, G], F32, tag=f"{nm}{g}", name=f"{nm}{g}")
                dsts.append(dst)
                for n in range(2):
                    cs = slice(512 * n, 512 * n + 512)
                    pp = ps_tile([128, 512], F32)
                    nc.tensor.matmul(pp[:], grp[g], qT[:, cs])
                    nc.vector.tensor_copy(dst[:, cs], pp[:])

        # ---- V natural + ones column, 32-wide head slots: Vp[t] [128, 8*32]
        # cols [32h,32h+16) = V_h, col 32h+16 = 1 (L1-denominator trick),
        # cols [32h+17,32h+32) = 0 so heads matmuls write full 32-row bands.
        vp_tiles = []
        for t in range(NT):
            vp = persist.tile([128, H * 32], BF16, tag=f"vp{t}")
            vp_tiles.append(vp)
            pv = ps_tile([128, HV], F32)
            nc.tensor.matmul(pv[:], qT[:, 128 * t : 128 * t + 128], WvAll[:])
            vp3 = vp[:].rearrange("p (h s) -> p h s", s=32)
            pv3 = pv[:].rearrange("p (h v) -> p h v", v=VD)
            nc.gpsimd.memset(vp3[:, :, VD:32], 0.0)
            nc.gpsimd.memset(vp3[:, :, VD : VD + 1], 1.0)
            nc.vector.tensor_copy(vp3[:, :, 0:VD], pv3[:])

        # ---- w' = w + (w==0) natural & transposed, bf16 ----
        # wpn[p, 512j+c] = w'[128j+p, c];  wpT[p, 512i+c] = w'[c, 128i+p]
        wn = persist.tile([128, 4 * U], F32, tag="wn")
        nc.sync.dma_start(
            wn[:].rearrange("p (j c) -> p j c", c=U),
            wb.rearrange("(j p) c -> p j c", p=128),
        )
        wpn = persist.tile([128, 4 * U], BF16, tag="wpn")
        nc.vector.scalar_tensor_tensor(
            wpn[:], wn[:], 0.0, wn[:], op0=ALU.is_equal, op1=ALU.add
        )
        wpT = persist.tile([128, 4 * U], BF16, tag="wpT")
        for i in range(4):
            for j in range(4):
                pb = ps_tile([128, 128], BF16)
                nc.tensor.transpose(
                    pb[:],
                    wpn[:, 512 * j + 128 * i : 512 * j + 128 * i + 128],
                    eye_b[:],
                )
                # pb[p, c] = w'[128j+c, 128i+p] -> wpT[p, 512i+128j+c]
                nc.vector.tensor_copy(
                    wpT[:, 512 * i + 128 * j : 512 * i + 128 * j + 128], pb[:]
                )

        # ---- mask natural bf16 (gpsimd converts u8->bf16) ----
        mb_tiles = []
        for t in range(NT):
            mu = work.tile([128, G], U8, tag="mu")
            nc.sync.dma_start(mu[:], maskb[128 * t : 128 * t + 128, :])
            mb = persist.tile([128, G], BF16, tag=f"mb{t}")
            mb_tiles.append(mb)
            nc.gpsimd.tensor_copy(mb[:], mu[:])

        # ---- P^T rows: PT[ki] [128, G] bf16 ----
        pt_tiles = []
        for ki in range(NT):
            ptile = persist.tile([128, G], BF16, tag=f"pt{ki}")
            pt_tiles.append(ptile)
            pm = ps_tile([128, G], BF16)
            for qj in range(NT):
                nc.tensor.transpose(
                    pm[:, 128 * qj : 128 * qj + 128],
                    mb_tiles[qj][:, 128 * ki : 128 * ki + 128],
                    eye_b[:],
                )
            if ki < 4:
                diag = slice(0, U)
                off = slice(U, G)
                wmul = wpT[:, 512 * ki : 512 * ki + 512]
            else:
                diag = slice(U, G)
                off = slice(0, U)
                wmul = wpn[:, 512 * (ki - 4) : 512 * (ki - 4) + 512]
            # diagonal half: (maskT == 0)
            nc.vector.tensor_scalar(
                ptile[:, diag], pm[:, diag], 0.0, None, op0=ALU.is_equal
            )
            # true-diagonal block: (maskT == eye)
            db = slice(128 * ki, 128 * ki + 128)
            nc.vector.scalar_tensor_tensor(
                ptile[:, db], pm[:, db], 0.0, eye_b[:],
                op0=ALU.bypass, op1=ALU.is_equal,
            )
            # off-diagonal half: (maskT == 0) * w'
            nc.vector.scalar_tensor_tensor(
                ptile[:, off], pm[:, off], 0.0, wmul,
                op0=ALU.is_equal, op1=ALU.mult,
            )

        # ---- main attention loop ----
        # Heads grouped 4 per PSUM tile: head slot m accumulates at
        # partitions [32m, 32m+17) (S row at 32m+16). Per head: all scores
        # matmuls first, then all heads matmuls (avoids PE tiling-mode
        # thrash); attnu chunks buffered until the heads pass.
        hstages = []
        for g, heads in enumerate(HGRP):
            phg = psum_h.tile([128, G], F32, tag="ph", name="ph")
            for m, h in enumerate(heads):
                rows = slice(32 * m, 32 * m + 32)
                attnus = []
                for c in range(NT):
                    ps = ps_tile([128, G], F32)
                    lhs_k = KT32[g][:][rows, 128 * c : 128 * c + 128]
                    for n in range(2):
                        cs = slice(512 * n, 512 * n + 512)
                        nc.tensor.matmul(
                            ps[:, cs], lhs_k, QT32[g][:][rows, cs]
                        )
                    compat = cpool.tile([128, G], BF16, tag="compat")
                    nc.scalar.activation(compat[:], ps[:], AF.Exp, scale=NORM)
                    attnu = apool.tile([128, G], BF16, tag="attnu")
                    # split the big multiply across DVE and GpSimd to
                    # balance per-engine byte traffic
                    eng = nc.gpsimd if c >= 5 else nc.vector
                    eng.tensor_tensor(
                        attnu[:], compat[:], pt_tiles[c][:], ALU.mult
                    )
                    attnus.append(attnu)
                vcol = 32 * h
                orow = slice(32 * m, 32 * m + 32)
                for c in range(NT):
                    vslice = vp_tiles[c][:][:, vcol : vcol + 32]
                    for n in range(2):
                        cs = slice(512 * n, 512 * n + 512)
                        nc.tensor.matmul(
                            phg[:][orow, cs], vslice, attnus[c][:, cs],
                            start=(c == 0), stop=(c == NT - 1),
                        )
            hst = persist.tile([128, G], F32, tag=f"hstage{g}", name=f"hstage{g}")
            hstages.append(hst)
            nrows = 32 * len(heads)
            nc.vector.tensor_copy(hst[0:nrows, :], phg[:][0:nrows, :])

        # ---- gather heads / S, normalize, project ----
        hAllT = persist.tile([HV, G], F32, tag="hAllT")
        for g, heads in enumerate(HGRP):
            for m, h in enumerate(heads):
                prow = 32 * m
                nc.sync.dma_start(
                    hAllT[VD * h : VD * h + VD, :],
                    hstages[g][prow : prow + VD, :],
                )
        # S rows -> Sall[h, :] (single-descriptor SBUF->SBUF DMAs), then
        # reciprocal straight on [8, 1024] (iterative divide, ~8.5us)
        sall = persist.tile([8, G], F32, tag="sall")
        for g, heads in enumerate(HGRP):
            for m, h in enumerate(heads):
                srow = 32 * m + VD
                nc.sync.dma_start(
                    sall[h : h + 1, :], hstages[g][srow : srow + 1, :]
                )
        rrows = persist.tile([8, G], F32, tag="rrows")
        nc.vector.reciprocal(rrows[:], sall[:])
        pr = pm_tile([HV, G], F32)
        for n in range(2):
            cs = slice(512 * n, 512 * n + 512)
            nc.tensor.matmul(pr[:, cs], brep[:], rrows[:, cs])
        hNorm = persist.tile([HV, G], F32, tag="hNorm")
        nc.vector.tensor_tensor(hNorm[:], hAllT[:], pr[:], ALU.mult)
        ostage = persist.tile([128, E], F32, tag="ostage")
        for t in range(NT):
            po = pm_tile([128, E], F32)
            nc.tensor.matmul(po[:], hNorm[:, 128 * t : 128 * t + 128], WoAll[:])
            ost = work.tile([128, E], F32, tag="ost")
            nc.vector.tensor_copy(ost[:], po[:])
            nc.sync.dma_start(outb[128 * t : 128 * t + 128, :], ost[:])
        del ostage


_CACHED = {}


def _get_nc(reps: int = 1):
    key = f"nc{reps}"
    if key not in _CACHED:
        nc = bacc.Bacc("TRN2", target_bir_lowering=False, debug=False)
        build_kernel(nc, reps)
        nc.compile()
        _CACHED[key] = nc
    return _CACHED[key]


HGRP_HOST = [(0, 1, 2), (3, 4, 5), (6, 7)]


def pack_weights(Wq, Wk, Wv, Wo):
    """Host-side layout prep of the small weight tensors (~256KB)."""
    def pack_qk(W):
        out = np.zeros((3, D, 128), np.float32)
        for g, heads in enumerate(HGRP_HOST):
            for m, h in enumerate(heads):
                out[g, :, 32 * m : 32 * m + KD] = W[h]
        return out

    return {
        "wqp": pack_qk(np.asarray(Wq, np.float32)),
        "wkp": pack_qk(np.asarray(Wk, np.float32)),
        "wvp": np.ascontiguousarray(
            np.asarray(Wv, np.float32).transpose(1, 0, 2).reshape(D, HV)
        ),
        "wop": np.ascontiguousarray(np.asarray(Wo, np.float32).reshape(HV, E)),
    }


def kernel(**inputs: np.ndarray) -> np.ndarray:
    q = np.ascontiguousarray(inputs["q"], dtype=np.float32)
    mask = np.ascontiguousarray(inputs["mask"]).view(np.uint8)
    weights = np.ascontiguousarray(inputs["weights"], dtype=np.float32)
    shared = pack_weights(
        inputs["W_query"], inputs["W_key"], inputs["W_val"], inputs["W_out"]
    )
    nc = _get_nc()
    in_maps = [
        {"qb": q[b], "maskb": mask[b], "wb": weights[b], **shared}
        for b in range(B)
    ]
    res = bass_utils.run_bass_kernel_spmd(
        nc, in_maps, core_ids=list(range(B)),
        trace=bool(int(os.environ.get("KERNEL_TRACE", "0"))),
    )
    out = np.stack([np.asarray(r["outb"]) for r in res.results])
    kernel.last_results = res
    return out.astype(np.float32)


if __name__ == "__main__":
    rng = np.random.default_rng(0)
    inputs = {
        "q": rng.standard_normal((B, G, D), dtype=np.float32),
        "mask": rng.integers(0, 2, (B, G, G)).astype(bool),
        "weights": rng.random((B, U, U), dtype=np.float32),
        "W_query": rng.random((H, D, KD), dtype=np.float32) - 0.5,
        "W_key": rng.random((H, D, KD), dtype=np.float32) - 0.5,
        "W_val": rng.random((H, D, VD), dtype=np.float32) - 0.5,
        "W_out": rng.random((H, KD, E), dtype=np.float32) - 0.5,
    }
    out = kernel(**inputs)
    print(out.shape, out.dtype)
